# revision 15
# baseline (speedup 1.0000x reference)
"""ChebConv (K=4) Trainium2 kernel — sparse scatter-matmul version.

Math (exactly matches the reference, which applies the spmm to `x` — not the
recurrence state — in every Chebyshev iteration):

    deg   = segment_sum(edge_weight, row)
    dinv  = deg^-1/2 (0 where deg <= 0)
    L[r,c]= sum over edges (r,c) of -2*dinv[r]*w*dinv[c];  L[i,i] += 2*fill
    Lx    = L @ x[b]                    (per batch)
    out   = x @ (W0 - W2) + Lx @ (W1 + 2*W2 + W3) + bias

Device strategy: the graph is 0.16% dense (160k edges + 10k self loops over
10000^2), so instead of densifying L, exploit sparsity.  Host buckets edges
by destination row into 8 cores x 10 windows of 128 rows, padding each
bucket to whole chunks of 128 edges.  For chunk ci of window w the device
computes

    Lx[w] += P_ci^T @ Xg_ci

where P_ci[e, j] = lap(e) if edge e's dst row (within the window) == j
else 0 (stationary operand, lap folded in host-side), and Xg_ci[e, :] =
x[:, src(e), :] (host-gathered source rows; 512 cols = 4 batches x 128
feats).  ~18 chunks/window -> ~180 matmuls/core instead of the dense 800,
streaming 24 MB of gathered rows + 6 MB of P per core.

The epilogue (transpose Lx to feature-major, apply the two weight matmuls,
add bias, DMA out) runs per window, lagged one window behind the scatter,
in bf16 — so the whole kernel is a single DMA-paced pipeline with no
serial tail.
"""

import numpy as np
import ml_dtypes

B = 4
N_NODES = 10000
F = 128
SELF_LOOP_FILL = -0.05
NCORES = 8
NPAD = 10240                 # 80 tiles of 128; divisible by 8 cores
MROWS = NPAD // NCORES       # 1280 output rows per core
MT = MROWS // 128            # 10 dst-row windows per core
BF = B * F                   # 512 moving columns

_state = {}


def _build_nc(cpw):
    from contextlib import ExitStack

    import concourse.bass as bass
    import concourse.bacc as bacc
    import concourse.tile as tile
    from concourse import mybir

    dt = mybir.dt
    nc = bacc.Bacc(
        "TRN2", target_bir_lowering=False, debug=False, num_devices=NCORES
    )

    ct = int(sum(cpw))
    pmat = nc.declare_dram_parameter(
        "pmat", [128, ct * 128], dt.float8e3, isOutput=False
    )
    xg = nc.declare_dram_parameter("xg", [128, ct * BF], dt.float8e3, isOutput=False)
    xt = nc.declare_dram_parameter("xt", [128, MT, BF], dt.bfloat16, isOutput=False)
    wa = nc.declare_dram_parameter("wa", [128, 128], dt.bfloat16, isOutput=False)
    wb = nc.declare_dram_parameter("wb", [128, 128], dt.bfloat16, isOutput=False)
    biasv = nc.declare_dram_parameter("biasv", [128, 1], dt.float32, isOutput=False)
    ident = nc.declare_dram_parameter("ident", [128, 128], dt.bfloat16, isOutput=False)
    out_t = nc.declare_dram_parameter(
        "out_t", [MT, 128, BF], dt.bfloat16, isOutput=True
    )

    with ExitStack() as ctx:
        tc = ctx.enter_context(tile.TileContext(nc))
        const = ctx.enter_context(tc.tile_pool(name="const", bufs=1))
        ppool = ctx.enter_context(tc.tile_pool(name="pchunk", bufs=6))
        xgpool = ctx.enter_context(tc.tile_pool(name="xgchunk", bufs=6))
        lxnpool = ctx.enter_context(tc.tile_pool(name="lxn", bufs=3))
        lxtpool = ctx.enter_context(tc.tile_pool(name="lxt", bufs=3))
        outpool = ctx.enter_context(tc.tile_pool(name="outstg", bufs=3))
        psA = ctx.enter_context(
            tc.tile_pool(name="psA", bufs=4, space=bass.MemorySpace.PSUM)
        )
        psT = ctx.enter_context(
            tc.tile_pool(name="psT", bufs=2, space=bass.MemorySpace.PSUM)
        )
        psB = ctx.enter_context(
            tc.tile_pool(name="psB", bufs=2, space=bass.MemorySpace.PSUM)
        )

        # constants + xt on the scalar HWDGE queue (off the streaming path)
        id_sb = const.tile([128, 128], dt.bfloat16, tag="ident")
        nc.scalar.dma_start(id_sb[:], ident[:])
        wa_sb = const.tile([128, 128], dt.bfloat16, tag="wa")
        nc.scalar.dma_start(wa_sb[:], wa[:])
        wb_sb = const.tile([128, 128], dt.bfloat16, tag="wb")
        nc.scalar.dma_start(wb_sb[:], wb[:])
        bias_sb = const.tile([128, 1], dt.float32, tag="bias")
        nc.scalar.dma_start(bias_sb[:], biasv[:])
        xt_sb = const.tile([128, MT, BF], dt.bfloat16, tag="xt")
        nc.scalar.dma_start(xt_sb[:], xt[:])

        # PE warmup without any DMA dependency: memset a tile, then dummy
        # matmuls so the HAM clock-gate opens before the first real chunk.
        wz = const.tile([128, 128], dt.bfloat16, tag="wz")
        nc.vector.memset(wz[:], 0.0)
        pwarm = psA.tile([128, 128], dt.float32, tag="ps", name="ps_warm")
        for i in range(36):
            nc.tensor.matmul(
                pwarm[:], wz[:], wz[:], start=(i == 0), stop=(i == 35)
            )

        ps1_tiles = [None] * MT
        woff = [0] * (MT + 1)
        for w in range(MT):
            woff[w + 1] = woff[w] + cpw[w]

        def load_window(w, groups):
            """DMA one window's P + Xg in `groups` pieces; return per-chunk
            (tile, local-chunk) views for the matmul loop."""
            cp = cpw[w]
            offp = woff[w] * 128
            offx = woff[w] * BF
            views = []
            bounds = [cp * gi // groups for gi in range(groups + 1)]
            for gi in range(groups):
                g0, g1 = bounds[gi], bounds[gi + 1]
                gl = g1 - g0
                if gl == 0:
                    continue
                pt = ppool.tile([128, gl * 128], dt.float8e3, tag="pt")
                nc.sync.dma_start(
                    pt[:], pmat[:, offp + g0 * 128 : offp + g1 * 128]
                )
                xgt = xgpool.tile([128, gl * BF], dt.float8e3, tag="xgt")
                nc.sync.dma_start(xgt[:], xg[:, offx + g0 * BF : offx + g1 * BF])
                views.extend((pt, xgt, k) for k in range(gl))
            return views

        def scatter_pair(wlist, viewlists):
            """Interleave the accumulating matmuls of two windows so
            consecutive PE ops hit alternating PSUM banks."""
            pss = []
            for w in wlist:
                ps1 = psA.tile([128, BF], dt.float32, tag="ps", name=f"ps1_{w}")
                ps1_tiles[w] = ps1
                pss.append(ps1)
            top = max(cpw[w] for w in wlist)
            for ci in range(top):
                for w, ps1, views in zip(wlist, pss, viewlists):
                    if ci >= cpw[w]:
                        continue
                    pt, xgt, k = views[ci]
                    nc.tensor.matmul(
                        ps1[:],
                        pt[:, k * 128 : (k + 1) * 128],
                        xgt[:, k * BF : (k + 1) * BF],
                        start=(ci == 0),
                        stop=(ci == cpw[w] - 1),
                    )

        lxt_tiles = [None] * MT

        def transp(w):
            # node-major Lx (psum fp32) -> sbuf bf16, then per-batch PE
            # transpose [node, f] -> [f, node]
            lxn = lxnpool.tile([128, BF], dt.bfloat16, tag="lxn")
            nc.vector.tensor_copy(lxn[:], ps1_tiles[w][:])
            ptr = psT.tile([128, BF], dt.bfloat16, tag="pst", name=f"pst_{w}")
            for b in range(B):
                nc.tensor.transpose(
                    ptr[:, b * 128 : (b + 1) * 128],
                    lxn[:, b * 128 : (b + 1) * 128],
                    id_sb[:],
                )
            lxt = lxtpool.tile([128, BF], dt.bfloat16, tag="lxt")
            nc.vector.tensor_copy(lxt[:], ptr[:])
            lxt_tiles[w] = lxt

        def phase2(w):
            # out_T = A^T x^T + B^T Lx^T (+bias); wa/wb are shared across
            # batches, so each term is a single 512-wide matmul
            lxt = lxt_tiles[w]
            ps2 = psB.tile([128, BF], dt.float32, tag="ps2", name=f"ps2_{w}")
            nc.tensor.matmul(ps2[:], wa_sb[:], xt_sb[:, w, :], start=True, stop=False)
            nc.tensor.matmul(ps2[:], wb_sb[:], lxt[:], start=False, stop=True)
            ot = outpool.tile([128, BF], dt.bfloat16, tag="ot")
            nc.scalar.activation(
                ot[:], ps2[:],
                mybir.ActivationFunctionType.Identity,
                bias=bias_sb[:],
            )
            nc.gpsimd.dma_start(out_t[w], ot[:])

        # software-pipelined by window pairs: the PE runs the interleaved
        # scatter of pair k, then transposes + feature matmuls of pair k-1,
        # so every cross-engine handoff (PSUM->SBUF copies on Vector) has a
        # scatter-block of slack.  Windows 0/1 are DMA'd in small groups so
        # the first matmuls start as soon as the first chunks land.
        for k in range(MT // 2):
            w0, w1 = 2 * k, 2 * k + 1
            views0 = load_window(w0, 3 if k == 0 else 1)
            views1 = load_window(w1, 2 if k == 0 else 1)
            scatter_pair([w0, w1], [views0, views1])
            if k > 0:
                transp(2 * k - 2)
                transp(2 * k - 1)
                phase2(2 * k - 2)
                phase2(2 * k - 1)
        transp(MT - 2)
        transp(MT - 1)
        phase2(MT - 2)
        phase2(MT - 1)

    return nc


def _get_nc(cpw):
    key = ("nc", tuple(cpw))
    if key not in _state:
        nc = _build_nc(cpw)
        nc.compile()
        _state[key] = nc
    return _state[key]


def _prep_inputs(x, edge_index, edge_weight, weight, bias):
    """Host-side graph preprocessing -> per-core device input maps."""
    bf16 = ml_dtypes.bfloat16
    row = np.asarray(edge_index[0], dtype=np.int64)
    col = np.asarray(edge_index[1], dtype=np.int64)
    w = np.asarray(edge_weight, dtype=np.float32)

    deg = np.bincount(row, weights=w.astype(np.float64), minlength=N_NODES)
    deg = deg.astype(np.float32)
    dinv = np.where(deg > 0, np.where(deg > 0, deg, 1.0) ** -0.5, 0.0).astype(
        np.float32
    )
    lap2 = (-2.0 * dinv[row] * w * dinv[col]).astype(np.float32)

    # append self loops as ordinary edges
    loops = np.arange(N_NODES, dtype=np.int64)
    rows_all = np.concatenate([row, loops])
    cols_all = np.concatenate([col, loops])
    laps_all = np.concatenate(
        [lap2, np.full(N_NODES, 2.0 * SELF_LOOP_FILL, np.float32)]
    )

    # bucket edges by (core, window) = destination row // 128, derive a
    # shared (SPMD) chunks-per-window schedule covering the fullest core
    g = rows_all // 128                                  # global window id
    cnt = np.bincount(g, minlength=NCORES * MT)
    cpw = np.maximum(
        (cnt.reshape(NCORES, MT).max(axis=0) + 127) // 128, 1
    ).astype(np.int64)
    ct = int(cpw.sum())                                  # chunks per core
    cum = np.zeros(MT, np.int64)
    cum[1:] = np.cumsum(cpw)[:-1]

    order = np.argsort(g, kind="stable")
    gs = g[order]
    starts = np.zeros(NCORES * MT + 1, np.int64)
    starts[1:] = np.cumsum(cnt)
    rank = np.arange(gs.size, dtype=np.int64) - starts[gs]
    cs = gs // MT
    slot = cum[gs % MT] * 128 + rank                     # slot within core
    p = slot % 128                                       # partition (edge lane)
    ci = slot // 128                                     # chunk within core
    j = rows_all[order] % 128                            # dst row within window

    srcs = np.zeros((NCORES, ct * 128), np.int64)
    srcs[cs, slot] = cols_all[order]
    pm = np.zeros((NCORES, 128, ct * 128), np.float32)
    pm[cs, p, ci * 128 + j] = laps_all[order]
    # P is sent as fp8 e3m4, pre-scaled by 8 to land in the normal range
    # (|lap| ~ 0.1-0.5); the /8 is folded into the B weight matrix below
    pmat = np.clip(pm * 8.0, -15.5, 15.5).astype(ml_dtypes.float8_e3m4)
    del pm

    xf = np.asarray(x, np.float32)
    W = np.asarray(weight, dtype=np.float32)
    A = W[0] - W[2]
    Bm = (W[1] + 2.0 * W[2] + W[3]) / 8.0    # absorbs the P fp8 pre-scale
    biasv = np.asarray(bias, dtype=np.float32).reshape(128, 1)
    identity = np.eye(128, dtype=np.float32)

    xn_pad = np.zeros((NPAD, B, F), np.float32)
    xn_pad[:N_NODES] = np.transpose(xf, (1, 0, 2))

    in_maps = []
    for c in range(NCORES):
        S = srcs[c].reshape(ct, 128)
        # xg[e_lane, ci, b*128+f] = x[b, src(ci, e_lane), f]  (fp8 e3m4)
        xgc = np.ascontiguousarray(
            np.transpose(xf[:, S, :], (2, 1, 0, 3)).reshape(128, ct * BF)
        ).astype(ml_dtypes.float8_e3m4)
        r0 = c * MROWS
        # xt[f, w, b*128+j] = x[b, r0 + w*128 + j, f]
        xtc = np.ascontiguousarray(
            xn_pad[r0 : r0 + MROWS]
            .reshape(MT, 128, B, F)
            .transpose(3, 0, 2, 1)
            .reshape(128, MT, BF)
        ).astype(bf16)
        in_maps.append(
            {
                "pmat": np.ascontiguousarray(pmat[c]),
                "xg": xgc,
                "xt": xtc,
                "wa": A.astype(bf16),
                "wb": Bm.astype(bf16),
                "biasv": biasv,
                "ident": identity.astype(bf16),
            }
        )
    return in_maps, tuple(int(v) for v in cpw)


def _ensure_ntff_hook():
    """Register the axon NTFF profiling hook if the image's antenv lacks it.

    The boot path degrades silently when ``antenv.axon_hooks`` is missing;
    recreate the tiny get/set holder and wire it to libaxon_pjrt.so so
    ``run_bass_kernel_spmd(trace=True)`` can capture NTFF profiles.
    """
    import sys
    import types

    try:
        from antenv.axon_hooks import get_axon_ntff_profile_hook  # noqa: F401

        return
    except ImportError:
        pass
    mod = types.ModuleType("antenv.axon_hooks")
    holder = {}
    mod.set_axon_ntff_profile_hook = lambda h: holder.__setitem__("h", h)
    mod.get_axon_ntff_profile_hook = lambda: holder.get("h")
    sys.modules["antenv.axon_hooks"] = mod
    import antenv

    antenv.axon_hooks = mod
    from trn_agent_boot.trn_boot import _ntff_profile_via_ctypes

    hook = _ntff_profile_via_ctypes("/opt/axon/libaxon_pjrt.so")
    if hook is not None:
        mod.set_axon_ntff_profile_hook(hook)


def kernel(x, edge_index, edge_weight, weight, bias):
    import os

    from concourse.bass_utils import run_bass_kernel_spmd

    x = np.asarray(x, dtype=np.float32)
    in_maps, cpw = _prep_inputs(x, edge_index, edge_weight, weight, bias)
    nc = _get_nc(cpw)
    trace = bool(int(os.environ.get("CHEB_TRACE", "0")))
    if trace:
        _ensure_ntff_hook()
    res = run_bass_kernel_spmd(nc, in_maps, list(range(NCORES)), trace=trace)
    _state["last_result"] = res
    # out_t[w, f, b*128+j] (per core) -> out[b, c*MROWS + w*128 + j, f]
    parts = []
    for c in range(NCORES):
        r = np.asarray(res.results[c]["out_t"], dtype=np.float32)
        parts.append(
            r.reshape(MT, 128, B, 128).transpose(2, 0, 3, 1).reshape(B, MROWS, F)
        )
    out = np.concatenate(parts, axis=1)[:, :N_NODES, :]
    return np.ascontiguousarray(out)


# revision 18
# speedup vs baseline: 1.1729x; 1.1729x over previous
"""ChebConv (K=4) Trainium2 kernel — sparse scatter-matmul version.

Math (exactly matches the reference, which applies the spmm to `x` — not the
recurrence state — in every Chebyshev iteration):

    deg   = segment_sum(edge_weight, row)
    dinv  = deg^-1/2 (0 where deg <= 0)
    L[r,c]= sum over edges (r,c) of -2*dinv[r]*w*dinv[c];  L[i,i] += 2*fill
    Lx    = L @ x[b]                    (per batch)
    out   = x @ (W0 - W2) + Lx @ (W1 + 2*W2 + W3) + bias

Device strategy: the graph is 0.16% dense (160k edges + 10k self loops over
10000^2), so instead of densifying L, exploit sparsity.  Host buckets edges
by destination row into 8 cores x 10 windows of 128 rows, padding each
bucket to whole chunks of 128 edges.  For chunk ci of window w the device
computes

    Lx[w] += P_ci^T @ Xg_ci

where P_ci[e, j] = lap(e) if edge e's dst row (within the window) == j
else 0 (stationary operand, lap folded in host-side), and Xg_ci[e, :] =
x[:, src(e), :] (host-gathered source rows; 512 cols = 4 batches x 128
feats).  ~18 chunks/window -> ~180 matmuls/core instead of the dense 800,
streaming 24 MB of gathered rows + 6 MB of P per core.

The epilogue (transpose Lx to feature-major, apply the two weight matmuls,
add bias, DMA out) runs per window, lagged one window behind the scatter,
in bf16 — so the whole kernel is a single DMA-paced pipeline with no
serial tail.
"""

import numpy as np
import ml_dtypes

B = 4
N_NODES = 10000
F = 128
SELF_LOOP_FILL = -0.05
NCORES = 8
NPAD = 10240                 # 80 tiles of 128; divisible by 8 cores
MROWS = NPAD // NCORES       # 1280 output rows per core
MT = MROWS // 128            # 10 dst-row windows per core
BF = B * F                   # 512 moving columns

_state = {}


def _build_nc(cpw):
    from contextlib import ExitStack

    import concourse.bass as bass
    import concourse.bacc as bacc
    import concourse.tile as tile
    from concourse import mybir

    dt = mybir.dt
    nc = bacc.Bacc(
        "TRN2", target_bir_lowering=False, debug=False, num_devices=NCORES
    )

    ct = int(sum(cpw))
    pmat = nc.declare_dram_parameter(
        "pmat", [128, ct * 128], dt.float8e3, isOutput=False
    )
    xg = nc.declare_dram_parameter("xg", [128, ct * BF], dt.float8e3, isOutput=False)
    xt = nc.declare_dram_parameter("xt", [128, MT, BF], dt.bfloat16, isOutput=False)
    wa = nc.declare_dram_parameter("wa", [128, 128], dt.bfloat16, isOutput=False)
    wb = nc.declare_dram_parameter("wb", [128, 128], dt.bfloat16, isOutput=False)
    biasv = nc.declare_dram_parameter("biasv", [128, 1], dt.float32, isOutput=False)
    ident = nc.declare_dram_parameter("ident", [128, 128], dt.bfloat16, isOutput=False)
    out_t = nc.declare_dram_parameter(
        "out_t", [MT, 128, BF], dt.bfloat16, isOutput=True
    )

    with ExitStack() as ctx:
        tc = ctx.enter_context(tile.TileContext(nc))
        const = ctx.enter_context(tc.tile_pool(name="const", bufs=1))
        ppool = ctx.enter_context(tc.tile_pool(name="pchunk", bufs=6))
        xgpool = ctx.enter_context(tc.tile_pool(name="xgchunk", bufs=6))
        lxnpool = ctx.enter_context(tc.tile_pool(name="lxn", bufs=3))
        lxtpool = ctx.enter_context(tc.tile_pool(name="lxt", bufs=3))
        outpool = ctx.enter_context(tc.tile_pool(name="outstg", bufs=3))
        psA = ctx.enter_context(
            tc.tile_pool(name="psA", bufs=3, space=bass.MemorySpace.PSUM)
        )
        psT = ctx.enter_context(
            tc.tile_pool(name="psT", bufs=2, space=bass.MemorySpace.PSUM)
        )
        psB = ctx.enter_context(
            tc.tile_pool(name="psB", bufs=2, space=bass.MemorySpace.PSUM)
        )

        # constants + xt on the scalar HWDGE queue (off the streaming path)
        id_sb = const.tile([128, 128], dt.bfloat16, tag="ident")
        nc.scalar.dma_start(id_sb[:], ident[:])
        wa_sb = const.tile([128, 128], dt.bfloat16, tag="wa")
        nc.scalar.dma_start(wa_sb[:], wa[:])
        wb_sb = const.tile([128, 128], dt.bfloat16, tag="wb")
        nc.scalar.dma_start(wb_sb[:], wb[:])
        bias_sb = const.tile([128, 1], dt.float32, tag="bias")
        nc.scalar.dma_start(bias_sb[:], biasv[:])
        xt_sb = const.tile([128, MT, BF], dt.bfloat16, tag="xt")
        nc.scalar.dma_start(xt_sb[:], xt[:])

        # PE warmup without any DMA dependency: memset a tile, then dummy
        # matmuls so the HAM clock-gate opens before the first real chunk.
        wz = const.tile([128, 128], dt.bfloat16, tag="wz")
        nc.vector.memset(wz[:], 0.0)
        pwarm = psA.tile([128, 128], dt.float32, tag="ps", name="ps_warm")
        for i in range(36):
            nc.tensor.matmul(
                pwarm[:], wz[:], wz[:], start=(i == 0), stop=(i == 35)
            )

        ps1_tiles = [None] * MT
        woff = [0] * (MT + 1)
        for w in range(MT):
            woff[w + 1] = woff[w] + cpw[w]

        def load_window(w, groups):
            """DMA one window's P + Xg in `groups` pieces; return per-chunk
            (tile, local-chunk) views for the matmul loop."""
            cp = cpw[w]
            offp = woff[w] * 128
            offx = woff[w] * BF
            views = []
            bounds = [cp * gi // groups for gi in range(groups + 1)]
            for gi in range(groups):
                g0, g1 = bounds[gi], bounds[gi + 1]
                gl = g1 - g0
                if gl == 0:
                    continue
                pt = ppool.tile([128, gl * 128], dt.float8e3, tag="pt")
                nc.sync.dma_start(
                    pt[:], pmat[:, offp + g0 * 128 : offp + g1 * 128]
                )
                xgt = xgpool.tile([128, gl * BF], dt.float8e3, tag="xgt")
                nc.sync.dma_start(xgt[:], xg[:, offx + g0 * BF : offx + g1 * BF])
                views.extend((pt, xgt, k) for k in range(gl))
            return views

        def scatter(w, views):
            """Accumulate one window's chunks into a PSUM bank; consecutive
            matmuls to the same bank stream back-to-back on the PE."""
            ps1 = psA.tile([128, BF], dt.float32, tag="ps", name=f"ps1_{w}")
            ps1_tiles[w] = ps1
            for ci in range(cpw[w]):
                pt, xgt, k = views[ci]
                nc.tensor.matmul(
                    ps1[:],
                    pt[:, k * 128 : (k + 1) * 128],
                    xgt[:, k * BF : (k + 1) * BF],
                    start=(ci == 0),
                    stop=(ci == cpw[w] - 1),
                )

        lxt_tiles = [None] * MT

        def transp(w):
            # node-major Lx (psum fp32) -> sbuf bf16, then per-batch PE
            # transpose [node, f] -> [f, node]
            lxn = lxnpool.tile([128, BF], dt.bfloat16, tag="lxn")
            nc.vector.tensor_copy(lxn[:], ps1_tiles[w][:])
            ptr = psT.tile([128, BF], dt.bfloat16, tag="pst", name=f"pst_{w}")
            for b in range(B):
                nc.tensor.transpose(
                    ptr[:, b * 128 : (b + 1) * 128],
                    lxn[:, b * 128 : (b + 1) * 128],
                    id_sb[:],
                )
            lxt = lxtpool.tile([128, BF], dt.bfloat16, tag="lxt")
            nc.vector.tensor_copy(lxt[:], ptr[:])
            lxt_tiles[w] = lxt

        def phase2(w):
            # out_T = A^T x^T + B^T Lx^T (+bias); wa/wb are shared across
            # batches, so each term is a single 512-wide matmul
            lxt = lxt_tiles[w]
            ps2 = psB.tile([128, BF], dt.float32, tag="ps2", name=f"ps2_{w}")
            nc.tensor.matmul(ps2[:], wa_sb[:], xt_sb[:, w, :], start=True, stop=False)
            nc.tensor.matmul(ps2[:], wb_sb[:], lxt[:], start=False, stop=True)
            ot = outpool.tile([128, BF], dt.bfloat16, tag="ot")
            nc.scalar.activation(
                ot[:], ps2[:],
                mybir.ActivationFunctionType.Identity,
                bias=bias_sb[:],
            )
            nc.gpsimd.dma_start(out_t[w], ot[:])

        # software-pipelined two windows deep: the PE runs scatter(w),
        # transposes of w-1, feature matmuls of w-2, so every cross-engine
        # handoff (PSUM->SBUF copies on Vector) has a window of slack.
        # Windows 0/1 are DMA'd in small groups so the first matmuls start
        # as soon as the first chunks land.
        for w in range(MT):
            groups = 3 if w == 0 else (2 if w == 1 else 1)
            views = load_window(w, groups)
            scatter(w, views)
            if w > 0:
                transp(w - 1)
            if w > 1:
                phase2(w - 2)
        transp(MT - 1)
        phase2(MT - 2)
        phase2(MT - 1)

    return nc


def _get_nc(cpw):
    key = ("nc", tuple(cpw))
    if key not in _state:
        nc = _build_nc(cpw)
        nc.compile()
        _state[key] = nc
    return _state[key]


def _prep_inputs(x, edge_index, edge_weight, weight, bias):
    """Host-side graph preprocessing -> per-core device input maps."""
    bf16 = ml_dtypes.bfloat16
    row = np.asarray(edge_index[0], dtype=np.int64)
    col = np.asarray(edge_index[1], dtype=np.int64)
    w = np.asarray(edge_weight, dtype=np.float32)

    deg = np.bincount(row, weights=w.astype(np.float64), minlength=N_NODES)
    deg = deg.astype(np.float32)
    dinv = np.where(deg > 0, np.where(deg > 0, deg, 1.0) ** -0.5, 0.0).astype(
        np.float32
    )
    lap2 = (-2.0 * dinv[row] * w * dinv[col]).astype(np.float32)

    # append self loops as ordinary edges
    loops = np.arange(N_NODES, dtype=np.int64)
    rows_all = np.concatenate([row, loops])
    cols_all = np.concatenate([col, loops])
    laps_all = np.concatenate(
        [lap2, np.full(N_NODES, 2.0 * SELF_LOOP_FILL, np.float32)]
    )

    # bucket edges by (core, window) = destination row // 128, derive a
    # shared (SPMD) chunks-per-window schedule covering the fullest core
    g = rows_all // 128                                  # global window id
    cnt = np.bincount(g, minlength=NCORES * MT)
    cpw = np.maximum(
        (cnt.reshape(NCORES, MT).max(axis=0) + 127) // 128, 1
    ).astype(np.int64)
    ct = int(cpw.sum())                                  # chunks per core
    cum = np.zeros(MT, np.int64)
    cum[1:] = np.cumsum(cpw)[:-1]

    order = np.argsort(g, kind="stable")
    gs = g[order]
    starts = np.zeros(NCORES * MT + 1, np.int64)
    starts[1:] = np.cumsum(cnt)
    rank = np.arange(gs.size, dtype=np.int64) - starts[gs]
    cs = gs // MT
    slot = cum[gs % MT] * 128 + rank                     # slot within core
    p = slot % 128                                       # partition (edge lane)
    ci = slot // 128                                     # chunk within core
    j = rows_all[order] % 128                            # dst row within window

    srcs = np.zeros((NCORES, ct * 128), np.int64)
    srcs[cs, slot] = cols_all[order]
    pm = np.zeros((NCORES, 128, ct * 128), np.float32)
    pm[cs, p, ci * 128 + j] = laps_all[order]
    # P is sent as fp8 e3m4, pre-scaled by 8 to land in the normal range
    # (|lap| ~ 0.1-0.5); the /8 is folded into the B weight matrix below
    pmat = np.clip(pm * 8.0, -15.5, 15.5).astype(ml_dtypes.float8_e3m4)
    del pm

    xf = np.asarray(x, np.float32)
    W = np.asarray(weight, dtype=np.float32)
    A = W[0] - W[2]
    Bm = (W[1] + 2.0 * W[2] + W[3]) / 8.0    # absorbs the P fp8 pre-scale
    biasv = np.asarray(bias, dtype=np.float32).reshape(128, 1)
    identity = np.eye(128, dtype=np.float32)

    xn_pad = np.zeros((NPAD, B, F), np.float32)
    xn_pad[:N_NODES] = np.transpose(xf, (1, 0, 2))

    in_maps = []
    for c in range(NCORES):
        S = srcs[c].reshape(ct, 128)
        # xg[e_lane, ci, b*128+f] = x[b, src(ci, e_lane), f]  (fp8 e3m4)
        xgc = np.ascontiguousarray(
            np.transpose(xf[:, S, :], (2, 1, 0, 3)).reshape(128, ct * BF)
        ).astype(ml_dtypes.float8_e3m4)
        r0 = c * MROWS
        # xt[f, w, b*128+j] = x[b, r0 + w*128 + j, f]
        xtc = np.ascontiguousarray(
            xn_pad[r0 : r0 + MROWS]
            .reshape(MT, 128, B, F)
            .transpose(3, 0, 2, 1)
            .reshape(128, MT, BF)
        ).astype(bf16)
        in_maps.append(
            {
                "pmat": np.ascontiguousarray(pmat[c]),
                "xg": xgc,
                "xt": xtc,
                "wa": A.astype(bf16),
                "wb": Bm.astype(bf16),
                "biasv": biasv,
                "ident": identity.astype(bf16),
            }
        )
    return in_maps, tuple(int(v) for v in cpw)


def _ensure_ntff_hook():
    """Register the axon NTFF profiling hook if the image's antenv lacks it.

    The boot path degrades silently when ``antenv.axon_hooks`` is missing;
    recreate the tiny get/set holder and wire it to libaxon_pjrt.so so
    ``run_bass_kernel_spmd(trace=True)`` can capture NTFF profiles.
    """
    import sys
    import types

    try:
        from antenv.axon_hooks import get_axon_ntff_profile_hook  # noqa: F401

        return
    except ImportError:
        pass
    mod = types.ModuleType("antenv.axon_hooks")
    holder = {}
    mod.set_axon_ntff_profile_hook = lambda h: holder.__setitem__("h", h)
    mod.get_axon_ntff_profile_hook = lambda: holder.get("h")
    sys.modules["antenv.axon_hooks"] = mod
    import antenv

    antenv.axon_hooks = mod
    from trn_agent_boot.trn_boot import _ntff_profile_via_ctypes

    hook = _ntff_profile_via_ctypes("/opt/axon/libaxon_pjrt.so")
    if hook is not None:
        mod.set_axon_ntff_profile_hook(hook)


def kernel(x, edge_index, edge_weight, weight, bias):
    import os

    from concourse.bass_utils import run_bass_kernel_spmd

    x = np.asarray(x, dtype=np.float32)
    in_maps, cpw = _prep_inputs(x, edge_index, edge_weight, weight, bias)
    nc = _get_nc(cpw)
    trace = bool(int(os.environ.get("CHEB_TRACE", "0")))
    if trace:
        _ensure_ntff_hook()
    res = run_bass_kernel_spmd(nc, in_maps, list(range(NCORES)), trace=trace)
    _state["last_result"] = res
    # out_t[w, f, b*128+j] (per core) -> out[b, c*MROWS + w*128 + j, f]
    parts = []
    for c in range(NCORES):
        r = np.asarray(res.results[c]["out_t"], dtype=np.float32)
        parts.append(
            r.reshape(MT, 128, B, 128).transpose(2, 0, 3, 1).reshape(B, MROWS, F)
        )
    out = np.concatenate(parts, axis=1)[:, :N_NODES, :]
    return np.ascontiguousarray(out)


# revision 25
# speedup vs baseline: 1.1816x; 1.0074x over previous
"""ChebConv (K=4) Trainium2 kernel — sparse scatter-matmul version.

Math (exactly matches the reference, which applies the spmm to `x` — not the
recurrence state — in every Chebyshev iteration):

    deg   = segment_sum(edge_weight, row)
    dinv  = deg^-1/2 (0 where deg <= 0)
    L[r,c]= sum over edges (r,c) of -2*dinv[r]*w*dinv[c];  L[i,i] += 2*fill
    Lx    = L @ x[b]                    (per batch)
    out   = x @ (W0 - W2) + Lx @ (W1 + 2*W2 + W3) + bias

Device strategy: the graph is 0.16% dense (160k edges + 10k self loops over
10000^2), so instead of densifying L, exploit sparsity.  Host buckets edges
by destination row into 8 cores x 10 windows of 128 rows, padding each
bucket to whole chunks of 128 edges.  For chunk ci of window w the device
computes

    Lx[w] += P_ci^T @ Xg_ci

where P_ci[e, j] = lap(e) if edge e's dst row (within the window) == j
else 0 (stationary operand, lap folded in host-side), and Xg_ci[e, :] =
x[:, src(e), :] (host-gathered source rows; 512 cols = 4 batches x 128
feats).  ~18 chunks/window -> ~180 matmuls/core instead of the dense 800,
streaming 24 MB of gathered rows + 6 MB of P per core.

The epilogue (transpose Lx to feature-major, apply the two weight matmuls,
add bias, DMA out) runs per window, lagged one window behind the scatter,
in bf16 — so the whole kernel is a single DMA-paced pipeline with no
serial tail.
"""

import numpy as np
import ml_dtypes

B = 4
N_NODES = 10000
F = 128
SELF_LOOP_FILL = -0.05
NCORES = 8
NPAD = 10240                 # 80 tiles of 128; divisible by 8 cores
MROWS = NPAD // NCORES       # 1280 output rows per core
MT = MROWS // 128            # 10 dst-row windows per core
BF = B * F                   # 512 moving columns

_state = {}


def _build_nc(cpw):
    from contextlib import ExitStack

    import concourse.bass as bass
    import concourse.bacc as bacc
    import concourse.tile as tile
    from concourse import mybir

    dt = mybir.dt
    nc = bacc.Bacc(
        "TRN2", target_bir_lowering=False, debug=False, num_devices=NCORES
    )

    ct = int(sum(cpw))
    pmat = nc.declare_dram_parameter(
        "pmat", [128, ct * 128], dt.float8e3, isOutput=False
    )
    xg = nc.declare_dram_parameter("xg", [128, ct * BF], dt.float8e3, isOutput=False)
    xt = nc.declare_dram_parameter("xt", [128, MT, BF], dt.bfloat16, isOutput=False)
    wa = nc.declare_dram_parameter("wa", [128, 128], dt.bfloat16, isOutput=False)
    wb = nc.declare_dram_parameter("wb", [128, 128], dt.bfloat16, isOutput=False)
    biasv = nc.declare_dram_parameter("biasv", [128, 1], dt.float32, isOutput=False)
    ident = nc.declare_dram_parameter("ident", [128, 128], dt.bfloat16, isOutput=False)
    out_t = nc.declare_dram_parameter(
        "out_t", [MT, 128, BF], dt.bfloat16, isOutput=True
    )

    with ExitStack() as ctx:
        tc = ctx.enter_context(tile.TileContext(nc))
        const = ctx.enter_context(tc.tile_pool(name="const", bufs=1))
        ppool = ctx.enter_context(tc.tile_pool(name="pchunk", bufs=6))
        xgpool = ctx.enter_context(tc.tile_pool(name="xgchunk", bufs=6))
        lxnpool = ctx.enter_context(tc.tile_pool(name="lxn", bufs=3))
        lxtpool = ctx.enter_context(tc.tile_pool(name="lxt", bufs=3))
        outpool = ctx.enter_context(tc.tile_pool(name="outstg", bufs=3))
        psA = ctx.enter_context(
            tc.tile_pool(name="psA", bufs=3, space=bass.MemorySpace.PSUM)
        )
        psT = ctx.enter_context(
            tc.tile_pool(name="psT", bufs=2, space=bass.MemorySpace.PSUM)
        )
        psB = ctx.enter_context(
            tc.tile_pool(name="psB", bufs=2, space=bass.MemorySpace.PSUM)
        )

        # constants + xt on the scalar HWDGE queue (off the streaming path)
        id_sb = const.tile([128, 128], dt.bfloat16, tag="ident")
        nc.scalar.dma_start(id_sb[:], ident[:])
        wa_sb = const.tile([128, 128], dt.bfloat16, tag="wa")
        nc.scalar.dma_start(wa_sb[:], wa[:])
        wb_sb = const.tile([128, 128], dt.bfloat16, tag="wb")
        nc.scalar.dma_start(wb_sb[:], wb[:])
        bias_sb = const.tile([128, 1], dt.float32, tag="bias")
        nc.scalar.dma_start(bias_sb[:], biasv[:])
        xt_sb = const.tile([128, MT, BF], dt.bfloat16, tag="xt")
        nc.scalar.dma_start(xt_sb[:], xt[:])

        # PE warmup without any DMA dependency: memset a tile, then dummy
        # matmuls so the HAM clock-gate opens before the first real chunk.
        wz = const.tile([128, 128], dt.bfloat16, tag="wz")
        nc.vector.memset(wz[:], 0.0)
        pwarm = psA.tile([128, 128], dt.float32, tag="ps", name="ps_warm")
        for i in range(20):
            nc.tensor.matmul(
                pwarm[:], wz[:], wz[:], start=(i == 0), stop=(i == 19)
            )

        ps1_tiles = [None] * MT
        woff = [0] * (MT + 1)
        for w in range(MT):
            woff[w + 1] = woff[w] + cpw[w]

        def load_window(w, groups):
            """DMA one window's P + Xg in `groups` pieces; return per-chunk
            (tile, local-chunk) views for the matmul loop."""
            cp = cpw[w]
            offp = woff[w] * 128
            offx = woff[w] * BF
            views = []
            bounds = [cp * gi // groups for gi in range(groups + 1)]
            for gi in range(groups):
                g0, g1 = bounds[gi], bounds[gi + 1]
                gl = g1 - g0
                if gl == 0:
                    continue
                pt = ppool.tile([128, gl * 128], dt.float8e3, tag="pt")
                nc.sync.dma_start(
                    pt[:], pmat[:, offp + g0 * 128 : offp + g1 * 128]
                )
                xgt = xgpool.tile([128, gl * BF], dt.float8e3, tag="xgt")
                nc.sync.dma_start(xgt[:], xg[:, offx + g0 * BF : offx + g1 * BF])
                views.extend((pt, xgt, k) for k in range(gl))
            return views

        def scatter(w, views):
            """Accumulate one window's chunks into a PSUM bank; consecutive
            matmuls to the same bank stream back-to-back on the PE."""
            ps1 = psA.tile([128, BF], dt.float32, tag="ps", name=f"ps1_{w}")
            ps1_tiles[w] = ps1
            for ci in range(cpw[w]):
                pt, xgt, k = views[ci]
                nc.tensor.matmul(
                    ps1[:],
                    pt[:, k * 128 : (k + 1) * 128],
                    xgt[:, k * BF : (k + 1) * BF],
                    start=(ci == 0),
                    stop=(ci == cpw[w] - 1),
                )

        lxt_tiles = [None] * MT

        def transp(w):
            # node-major Lx (psum fp32) -> sbuf bf16, then per-batch PE
            # transpose [node, f] -> [f, node]
            lxn = lxnpool.tile([128, BF], dt.bfloat16, tag="lxn")
            nc.vector.tensor_copy(lxn[:], ps1_tiles[w][:])
            ptr = psT.tile([128, BF], dt.bfloat16, tag="pst", name=f"pst_{w}")
            for b in range(B):
                nc.tensor.transpose(
                    ptr[:, b * 128 : (b + 1) * 128],
                    lxn[:, b * 128 : (b + 1) * 128],
                    id_sb[:],
                )
            lxt = lxtpool.tile([128, BF], dt.bfloat16, tag="lxt")
            nc.vector.tensor_copy(lxt[:], ptr[:])
            lxt_tiles[w] = lxt

        def phase2(w):
            # out_T = A^T x^T + B^T Lx^T (+bias); wa/wb are shared across
            # batches, so each term is a single 512-wide matmul
            lxt = lxt_tiles[w]
            ps2 = psB.tile([128, BF], dt.float32, tag="ps2", name=f"ps2_{w}")
            nc.tensor.matmul(ps2[:], wa_sb[:], xt_sb[:, w, :], start=True, stop=False)
            nc.tensor.matmul(ps2[:], wb_sb[:], lxt[:], start=False, stop=True)
            ot = outpool.tile([128, BF], dt.bfloat16, tag="ot")
            nc.scalar.activation(
                ot[:], ps2[:],
                mybir.ActivationFunctionType.Identity,
                bias=bias_sb[:],
            )
            nc.gpsimd.dma_start(out_t[w], ot[:])

        # software-pipelined two windows deep: the PE runs scatter(w),
        # transposes of w-1, feature matmuls of w-2, so every cross-engine
        # handoff (PSUM->SBUF copies on Vector) has a window of slack.
        # Windows 0/1 are DMA'd in small groups so the first matmuls start
        # as soon as the first chunks land.
        for w in range(MT):
            groups = 3 if w == 0 else (2 if w == 1 else 1)
            views = load_window(w, groups)
            scatter(w, views)
            if w > 0:
                transp(w - 1)
            if w > 1:
                phase2(w - 2)
        transp(MT - 1)
        phase2(MT - 2)    # lxt(MT-2) copy landed during transp(MT-1)
        phase2(MT - 1)

    return nc


def _get_nc(cpw):
    key = ("nc", tuple(cpw))
    if key not in _state:
        nc = _build_nc(cpw)
        nc.compile()
        _state[key] = nc
    return _state[key]


def _prep_inputs(x, edge_index, edge_weight, weight, bias):
    """Host-side graph preprocessing -> per-core device input maps."""
    bf16 = ml_dtypes.bfloat16
    row = np.asarray(edge_index[0], dtype=np.int64)
    col = np.asarray(edge_index[1], dtype=np.int64)
    w = np.asarray(edge_weight, dtype=np.float32)

    deg = np.bincount(row, weights=w.astype(np.float64), minlength=N_NODES)
    deg = deg.astype(np.float32)
    dinv = np.where(deg > 0, np.where(deg > 0, deg, 1.0) ** -0.5, 0.0).astype(
        np.float32
    )
    lap2 = (-2.0 * dinv[row] * w * dinv[col]).astype(np.float32)

    # append self loops as ordinary edges
    loops = np.arange(N_NODES, dtype=np.int64)
    rows_all = np.concatenate([row, loops])
    cols_all = np.concatenate([col, loops])
    laps_all = np.concatenate(
        [lap2, np.full(N_NODES, 2.0 * SELF_LOOP_FILL, np.float32)]
    )

    # bucket edges by (core, window) = destination row // 128.  The SPMD
    # schedule (chunks per window slot) is shared across cores, so assign
    # each core's windows to slots in descending-count order — the max over
    # cores of rank-matched counts is much tighter than positional matching
    g = rows_all // 128                                  # global window id
    cnt = np.bincount(g, minlength=NCORES * MT).reshape(NCORES, MT)
    # ascending: smallest windows stream first, shortening the pipe ramp
    perm = np.argsort(cnt, axis=1, kind="stable")        # slot s -> window id
    inv_perm = np.argsort(perm, axis=1)                  # window id -> slot
    cnt_slot = np.take_along_axis(cnt, perm, axis=1)     # counts by slot
    cpw = np.maximum((cnt_slot.max(axis=0) + 127) // 128, 1).astype(np.int64)
    ct = int(cpw.sum())                                  # chunks per core
    cum = np.zeros(MT, np.int64)
    cum[1:] = np.cumsum(cpw)[:-1]

    order = np.argsort(g, kind="stable")
    gs = g[order]
    starts = np.zeros(NCORES * MT + 1, np.int64)
    starts[1:] = np.cumsum(cnt.reshape(-1))
    rank = np.arange(gs.size, dtype=np.int64) - starts[gs]
    cs = gs // MT
    sl = inv_perm[cs, gs % MT]                           # slot of this bucket
    slot = cum[sl] * 128 + rank                          # edge slot within core
    p = slot % 128                                       # partition (edge lane)
    ci = slot // 128                                     # chunk within core
    j = rows_all[order] % 128                            # dst row within window

    srcs = np.zeros((NCORES, ct * 128), np.int64)
    srcs[cs, slot] = cols_all[order]
    pm = np.zeros((NCORES, 128, ct * 128), np.float32)
    pm[cs, p, ci * 128 + j] = laps_all[order]
    # P is sent as fp8 e3m4, pre-scaled by 8 to land in the normal range
    # (|lap| ~ 0.1-0.5); the /8 is folded into the B weight matrix below
    pmat = np.clip(pm * 8.0, -15.5, 15.5).astype(ml_dtypes.float8_e3m4)
    del pm

    xf = np.asarray(x, np.float32)
    W = np.asarray(weight, dtype=np.float32)
    A = W[0] - W[2]
    Bm = (W[1] + 2.0 * W[2] + W[3]) / 8.0    # absorbs the P fp8 pre-scale
    biasv = np.asarray(bias, dtype=np.float32).reshape(128, 1)
    identity = np.eye(128, dtype=np.float32)

    xn_pad = np.zeros((NPAD, B, F), np.float32)
    xn_pad[:N_NODES] = np.transpose(xf, (1, 0, 2))

    in_maps = []
    for c in range(NCORES):
        S = srcs[c].reshape(ct, 128)
        # xg[e_lane, ci, b*128+f] = x[b, src(ci, e_lane), f]  (fp8 e3m4)
        xgc = np.ascontiguousarray(
            np.transpose(xf[:, S, :], (2, 1, 0, 3)).reshape(128, ct * BF)
        ).astype(ml_dtypes.float8_e3m4)
        r0 = c * MROWS
        # xt[f, s, b*128+j] = x[b, r0 + perm[c][s]*128 + j, f]
        xtc = np.ascontiguousarray(
            xn_pad[r0 : r0 + MROWS]
            .reshape(MT, 128, B, F)[perm[c]]
            .transpose(3, 0, 2, 1)
            .reshape(128, MT, BF)
        ).astype(bf16)
        in_maps.append(
            {
                "pmat": np.ascontiguousarray(pmat[c]),
                "xg": xgc,
                "xt": xtc,
                "wa": A.astype(bf16),
                "wb": Bm.astype(bf16),
                "biasv": biasv,
                "ident": identity.astype(bf16),
            }
        )
    return in_maps, tuple(int(v) for v in cpw), perm


def _ensure_ntff_hook():
    """Register the axon NTFF profiling hook if the image's antenv lacks it.

    The boot path degrades silently when ``antenv.axon_hooks`` is missing;
    recreate the tiny get/set holder and wire it to libaxon_pjrt.so so
    ``run_bass_kernel_spmd(trace=True)`` can capture NTFF profiles.
    """
    import sys
    import types

    try:
        from antenv.axon_hooks import get_axon_ntff_profile_hook  # noqa: F401

        return
    except ImportError:
        pass
    mod = types.ModuleType("antenv.axon_hooks")
    holder = {}
    mod.set_axon_ntff_profile_hook = lambda h: holder.__setitem__("h", h)
    mod.get_axon_ntff_profile_hook = lambda: holder.get("h")
    sys.modules["antenv.axon_hooks"] = mod
    import antenv

    antenv.axon_hooks = mod
    from trn_agent_boot.trn_boot import _ntff_profile_via_ctypes

    hook = _ntff_profile_via_ctypes("/opt/axon/libaxon_pjrt.so")
    if hook is not None:
        mod.set_axon_ntff_profile_hook(hook)


def kernel(x, edge_index, edge_weight, weight, bias):
    import os

    from concourse.bass_utils import run_bass_kernel_spmd

    x = np.asarray(x, dtype=np.float32)
    in_maps, cpw, perm = _prep_inputs(x, edge_index, edge_weight, weight, bias)
    nc = _get_nc(cpw)
    trace = bool(int(os.environ.get("CHEB_TRACE", "0")))
    if trace:
        _ensure_ntff_hook()
    res = run_bass_kernel_spmd(nc, in_maps, list(range(NCORES)), trace=trace)
    _state["last_result"] = res
    # out_t[s, f, b*128+j] (per core) -> out[b, c*MROWS + perm[c][s]*128 + j, f]
    parts = []
    for c in range(NCORES):
        r = np.asarray(res.results[c]["out_t"], dtype=np.float32)
        arr = r.reshape(MT, 128, B, 128).transpose(2, 0, 3, 1)  # (B, s, j, F)
        oc = np.empty((B, MT, 128, F), np.float32)
        oc[:, perm[c]] = arr
        parts.append(oc.reshape(B, MROWS, F))
    out = np.concatenate(parts, axis=1)[:, :N_NODES, :]
    return np.ascontiguousarray(out)
